# revision 12
# baseline (speedup 1.0000x reference)
"""ComAttention Trainium2 kernel.

Strategy (B=8 sharded 1 batch element per core, 8 cores, no collectives):

The three 1x1 "fusion" convs over the head-channel dim have no nonlinearity
between them, so they collapse to a single linear functional over head
channels:  h3[i,j] = sum_c g[c] * s_c[i,j] + C  with g = fw3@fw2@fw1 and
C = fw3@fw2@fb1 + fw3@fb2 + fb3.  Folding g[c]*SCALE into Wq gives
h3 = (X@Wqg + bqg) @ (X@Wk + bk)^T + C -- one more attention-score matmul.
The gate is s>0.5 <=> h3>0 (mask values only matter via ==0 in the
reference), with a sign margin ~0.018 >> fp32r noise ~1e-5.

Everything on-device is computed in a transposed convention
(out^T = W^T @ X^T) so the PE contraction dim is always the partition dim
and no on-device transposes are needed; the host pre-transposes the inputs
and post-transposes the outputs.

Masked softmax without max subtraction (scores are O(1)):
E = exp(S)*mask ; all-masked columns handled exactly via E_aug = E + U
(U[i]=1 iff column i fully masked), Z from an appended ones-column in V,
giving the reference's uniform-1/L rows bit-exactly for fully-masked rows.

Matmuls run as float32r (TF32-like, ~1.5e-4 rel, 4x the fp32 rate).
"""

import numpy as np
from contextlib import ExitStack

import concourse.bass as bass
import concourse.tile as tile
from concourse import bacc, mybir
from concourse.bass_utils import run_bass_kernel_spmd

B, L, H, D, DK = 8, 512, 16, 1024, 64
MT = D // 128  # 8 partition tiles over D
LT = L // 128  # 4 partition tiles over L
F32 = mybir.dt.float32
F32R = mybir.dt.float32r
AF = mybir.ActivationFunctionType
ALU = mybir.AluOpType

_BIG_W = ["wqg", "wk", "pwq", "pwk", "nwq", "nwk", "pwo", "nwo", "wvp", "wvn", "wgp"]
_BIASES = ["bqg", "bk", "pbq", "pbk", "pbo", "nbq", "nbk", "nbo", "bvp", "bvn", "bgp"]

_cache = {}


def _build():
    if "nc" in _cache:
        return _cache["nc"]

    nc = bacc.Bacc("TRN2", target_bir_lowering=False, debug=False, num_devices=8)

    d_xT = nc.dram_tensor("xT", [D, L], F32R, kind="ExternalInput").ap()
    d_dmT = nc.dram_tensor("dmT", [L, L], F32, kind="ExternalInput").ap()
    d_negc = nc.dram_tensor("negc", [128, 1], F32, kind="ExternalInput").ap()
    d_ones = nc.dram_tensor("ones", [128], F32R, kind="ExternalInput").ap()
    d_w = {
        n: nc.dram_tensor(n, [MT, MT, 128, 128], F32R, kind="ExternalInput").ap()
        for n in _BIG_W
    }
    d_wv = {
        n: nc.dram_tensor(n, [2, MT, 128, 512], F32R, kind="ExternalInput").ap()
        for n in ("pwv", "nwv")
    }
    d_b = {n: nc.dram_tensor(n, [D], F32, kind="ExternalInput").ap() for n in _BIASES}
    d_bvr = {
        n: nc.dram_tensor(n, [1, D], F32R, kind="ExternalInput").ap()
        for n in ("pbvr", "nbvr")
    }
    d_pmT = nc.dram_tensor("pmT", [H, L, L], F32R, kind="ExternalOutput").ap()
    d_nmT = nc.dram_tensor("nmT", [H, L, L], F32R, kind="ExternalOutput").ap()
    d_outT = nc.dram_tensor("outT", [D, L], F32, kind="ExternalOutput").ap()

    with tile.TileContext(nc) as tc, ExitStack() as ctx:
        P0 = ctx.enter_context(tc.tile_pool(name="persist", bufs=1))
        psProj0 = ctx.enter_context(tc.tile_pool(name="psP0", bufs=2, space="PSUM"))

        ones_row = P0.tile([1, 128], F32R)
        nc.sync.dma_start(ones_row[:], d_ones.rearrange("(a k) -> a k", a=1))
        ones_col = P0.tile([128, 1], F32R)
        nc.sync.dma_start(ones_col[:], d_ones.rearrange("(k a) -> k a", a=1))
        negc_sb = P0.tile([128, 1], F32)
        nc.sync.dma_start(negc_sb[:], d_negc[:])
        bias_sb = {}
        for n in _BIASES:
            t = P0.tile([128, MT], F32, tag=f"b_{n}", name=f"b_{n}")
            nc.sync.dma_start(t[:], d_b[n].rearrange("(m p) -> p m", p=128))
            bias_sb[n] = t
        bvr_sb = {}
        for n in ("pbvr", "nbvr"):
            t = P0.tile([1, D], F32R, tag=f"r_{n}", name=f"r_{n}")
            nc.sync.dma_start(t[:], d_bvr[n][:])
            bvr_sb[n] = t

        IpT = P0.tile([128, LT * 512], F32)
        InT = P0.tile([128, LT * 512], F32)
        oTp = P0.tile([128, MT * 512], F32R)
        oTn = P0.tile([128, MT * 512], F32R)

        def proj_T(pool, pspool, dst, wname, bname, src):
            """dst[:, m*512:+512] (f32r [128 dout, 512 L] per m) = W^T @ src + b."""
            for m in range(MT):
                wst = pool.tile([128, 1024], F32R, tag="wst", bufs=3, name=f"wst_{wname}{m}")
                nc.sync.dma_start(
                    wst[:].rearrange("p (k c) -> p k c", k=MT),
                    d_w[wname][m].rearrange("k p c -> p k c"),
                )
                ps = pspool.tile([128, 512], F32, tag="proj", bufs=2, name=f"ps_{wname}{m}")
                for k in range(MT):
                    nc.tensor.matmul(
                        ps[:],
                        wst[:, k * 128 : (k + 1) * 128],
                        src[:, k * 512 : (k + 1) * 512],
                        start=(k == 0),
                        stop=(k == MT - 1),
                    )
                nc.scalar.activation(
                    dst[:, m * 512 : (m + 1) * 512],
                    ps[:],
                    AF.Identity,
                    bias=bias_sb[bname][:, m : m + 1],
                )

        with tc.tile_pool(name="pXT", bufs=1) as P1:
            xt = P1.tile([128, MT * 512], F32R)
            for m in range(MT):
                nc.sync.dma_start(
                    xt[:, m * 512 : (m + 1) * 512], d_xT[m * 128 : (m + 1) * 128, :]
                )

            # ---- phase A: score-branch projections + gate masks ----
            with tc.tile_pool(name="pA", bufs=1) as P2:
                kt = P2.tile([128, MT * 512], F32R)
                qgt = P2.tile([128, MT * 512], F32R)
                proj_T(P2, psProj0, kt, "wk", "bk", xt)
                proj_T(P2, psProj0, qgt, "wqg", "bqg", xt)
                for j in range(LT):
                    hps = psProj0.tile([128, 512], F32, tag="proj", bufs=2, name=f"hps{j}")
                    for m in range(MT):
                        nc.tensor.matmul(
                            hps[:],
                            kt[:, m * 512 + j * 128 : m * 512 + j * 128 + 128],
                            qgt[:, m * 512 : (m + 1) * 512],
                            start=(m == 0),
                            stop=(m == MT - 1),
                        )
                    dmj = P2.tile([128, 512], F32, tag="dmj", bufs=2, name=f"dmj{j}")
                    nc.sync.dma_start(dmj[:], d_dmT[j * 128 : (j + 1) * 128, :])
                    dmi = P2.tile([128, 512], F32, tag="dmi", bufs=2, name=f"dmi{j}")
                    nc.vector.tensor_scalar(dmi[:], dmj[:], 0.0, None, ALU.not_equal)
                    gp = P2.tile([128, 512], F32, tag="gp", bufs=2, name=f"gp{j}")
                    nc.vector.tensor_scalar(
                        gp[:], hps[:], negc_sb[:, 0:1], None, ALU.is_gt
                    )
                    jj = slice(j * 512, (j + 1) * 512)
                    nc.vector.tensor_tensor(IpT[:, jj], gp[:], dmi[:], op=ALU.mult)
                    nc.vector.tensor_sub(InT[:, jj], dmi[:], IpT[:, jj])

            def proj_V(pool, vnat, wvname, bvrname):
                """V natural [128 j-rows, LT*1024]: j-block at cols j*1024."""
                for nh in range(2):
                    vps = [
                        pool_ps_v.tile([128, 512], F32, tag=f"vj{j}", bufs=1, name=f"v{wvname}{nh}{j}")
                        for j in range(LT)
                    ]
                    for k in range(MT):
                        vst = pool.tile([128, 512], F32R, tag="vst", bufs=3, name=f"vst{wvname}{nh}{k}")
                        nc.sync.dma_start(vst[:], d_wv[wvname][nh, k])
                        for j in range(LT):
                            nc.tensor.matmul(
                                vps[j][:],
                                xt[:, k * 512 + j * 128 : k * 512 + j * 128 + 128],
                                vst[:],
                                start=(k == 0),
                                stop=False,
                            )
                    for j in range(LT):
                        nc.tensor.matmul(
                            vps[j][:],
                            ones_row[:],
                            bvr_sb[bvrname][0:1, nh * 512 : (nh + 1) * 512],
                            start=False,
                            stop=True,
                        )
                        nc.vector.tensor_copy(
                            vnat[:, j * 1024 + nh * 512 : j * 1024 + (nh + 1) * 512],
                            vps[j][:],
                        )

            def attention(pool, psA, qt, kt_, vnat, Imask, d_m, oT):
                for t in range(H // 2):
                    for hh in range(2):
                        h = 2 * t + hh
                        mh, oh = t, hh * 64
                        avps = psA.tile([128, 512], F32, tag="av", bufs=2, name=f"avp{h}")
                        Esb = pool.tile([128, LT * 512], F32R, tag="E", bufs=2, name=f"E{h}")
                        for j in range(LT):
                            sps = psA.tile([128, 512], F32, tag="mm", bufs=2, name=f"s{h}{j}")
                            nc.tensor.matmul(
                                sps[:],
                                kt_[oh : oh + 64, mh * 512 + j * 128 : mh * 512 + j * 128 + 128],
                                qt[oh : oh + 64, mh * 512 : (mh + 1) * 512],
                                start=True,
                                stop=True,
                            )
                            etmp = pool.tile([128, 512], F32, tag="etmp", bufs=3, name=f"et{h}{j}")
                            nc.scalar.activation(etmp[:], sps[:], AF.Exp)
                            nc.gpsimd.tensor_tensor(
                                Esb[:, j * 512 : (j + 1) * 512],
                                etmp[:],
                                Imask[:, j * 512 : (j + 1) * 512],
                                op=ALU.mult,
                            )
                        zps = psA.tile([128, 512], F32, tag="zb", bufs=2, name=f"z{h}")
                        for j in range(LT):
                            nc.tensor.matmul(
                                zps[0:1, :],
                                ones_col[:],
                                Esb[:, j * 512 : (j + 1) * 512],
                                start=(j == 0),
                                stop=(j == LT - 1),
                            )
                        u_row = P0.tile([1, 512], F32R, tag="u", bufs=2, name=f"u{h}")
                        nc.vector.tensor_scalar(
                            u_row[:], zps[0:1, :], 0.0, None, ALU.is_equal
                        )
                        zfix = P0.tile([1, 512], F32, tag="zf", bufs=2, name=f"zf{h}")
                        nc.vector.scalar_tensor_tensor(
                            zfix[:], u_row[:], 512.0, zps[0:1, :], ALU.mult, ALU.add
                        )
                        rrow = P0.tile([1, 512], F32R, tag="rr", bufs=2, name=f"rr{h}")
                        with nc.allow_low_precision(
                            reason="f32r R row: TF32 rounding of 1/Z within tolerance"
                        ):
                            nc.vector.reciprocal(rrow[:], zfix[:])
                        ups = psA.tile([128, 512], F32, tag="zb", bufs=2, name=f"ub{h}")
                        nc.tensor.matmul(ups[:], ones_row[:], u_row[:], start=True, stop=True)
                        rps = psA.tile([128, 512], F32, tag="zb", bufs=2, name=f"rb{h}")
                        nc.tensor.matmul(rps[:], ones_row[:], rrow[:], start=True, stop=True)
                        for j in range(LT):
                            jj = slice(j * 512, (j + 1) * 512)
                            nc.vector.tensor_add(Esb[:, jj], Esb[:, jj], ups[:].bitcast(F32R))
                            nc.vector.tensor_mul(Esb[:, jj], Esb[:, jj], rps[:].bitcast(F32R))
                            nc.sync.dma_start(
                                d_m[h, j * 128 : (j + 1) * 128, :], Esb[:, jj]
                            )
                            nc.tensor.matmul(
                                avps[0:64, :],
                                vnat[:, j * 1024 + h * 64 : j * 1024 + h * 64 + 64],
                                Esb[:, jj],
                                start=(j == 0),
                                stop=(j == LT - 1),
                            )
                        otmp = pool.tile([64, 512], F32R, tag="otmp", bufs=2, name=f"ot{h}")
                        nc.vector.tensor_copy(otmp[:], avps[0:64, :])
                        nc.sync.dma_start(
                            oT[oh : oh + 64, mh * 512 : (mh + 1) * 512], otmp[:]
                        )

            # ---- phase B: p branch ----
            with tc.tile_pool(name="pB", bufs=1) as P3:
                pqt = P3.tile([128, MT * 512], F32R)
                pkt = P3.tile([128, MT * 512], F32R)
                with tc.tile_pool(name="psB0", bufs=2, space="PSUM") as psB0:
                    proj_T(P3, psB0, pqt, "pwq", "pbq", xt)
                    proj_T(P3, psB0, pkt, "pwk", "pbk", xt)
                vnat_p = P3.tile([128, LT * 1024], F32R)
                with tc.tile_pool(name="psBV", bufs=1, space="PSUM") as pool_ps_v:
                    proj_V(P3, vnat_p, "pwv", "pbvr")
                with tc.tile_pool(name="psBA", bufs=1, space="PSUM") as psA:
                    attention(P3, psA, pqt, pkt, vnat_p, IpT, d_pmT, oTp)

            # ---- phase C: n branch ----
            with tc.tile_pool(name="pC", bufs=1) as P4:
                nqt = P4.tile([128, MT * 512], F32R)
                nkt = P4.tile([128, MT * 512], F32R)
                with tc.tile_pool(name="psC0", bufs=2, space="PSUM") as psC0:
                    proj_T(P4, psC0, nqt, "nwq", "nbq", xt)
                    proj_T(P4, psC0, nkt, "nwk", "nbk", xt)
                vnat_n = P4.tile([128, LT * 1024], F32R)
                with tc.tile_pool(name="psCV", bufs=1, space="PSUM") as pool_ps_v:
                    proj_V(P4, vnat_n, "nwv", "nbvr")
                with tc.tile_pool(name="psCA", bufs=1, space="PSUM") as psA:
                    attention(P4, psA, nqt, nkt, vnat_n, InT, d_nmT, oTn)

        # ---- phase D: output projections + gated combine ----
        with tc.tile_pool(name="pD", bufs=1) as P5, tc.tile_pool(
            name="psD", bufs=2, space="PSUM"
        ) as psD:
            pt = P5.tile([128, MT * 512], F32R)
            nt = P5.tile([128, MT * 512], F32R)
            proj_T(P5, psD, pt, "pwo", "pbo", oTp)
            proj_T(P5, psD, nt, "nwo", "nbo", oTn)

            for m in range(MT):
                mm = slice(m * 512, (m + 1) * 512)
                outs = {}
                for nm_, wname, bname, src in (
                    ("vp", "wvp", "bvp", pt),
                    ("vn", "wvn", "bvn", nt),
                    ("ep", "wgp", "bgp", pt),
                    ("en", "wgp", "bgp", nt),
                ):
                    wst = P5.tile([128, 1024], F32R, tag=f"wst_{nm_}", bufs=2, name=f"w{nm_}{m}")
                    nc.sync.dma_start(
                        wst[:].rearrange("p (k c) -> p k c", k=MT),
                        d_w[wname][m].rearrange("k p c -> p k c"),
                    )
                    ps = psD.tile([128, 512], F32, tag=f"proj", bufs=2, name=f"psd{nm_}{m}")
                    for k in range(MT):
                        nc.tensor.matmul(
                            ps[:],
                            wst[:, k * 128 : (k + 1) * 128],
                            src[:, k * 512 : (k + 1) * 512],
                            start=(k == 0),
                            stop=(k == MT - 1),
                        )
                    t = P5.tile([128, 512], F32, tag=f"o_{nm_}", bufs=2, name=f"t{nm_}{m}")
                    nc.scalar.activation(
                        t[:], ps[:], AF.Identity, bias=bias_sb[bname][:, m : m + 1]
                    )
                    outs[nm_] = t
                d_t = P5.tile([128, 512], F32, tag="gt1", bufs=2, name=f"d{m}")
                nc.vector.tensor_sub(d_t[:], outs["ep"][:], outs["en"][:])
                s_t = P5.tile([128, 512], F32, tag="gt2", bufs=2, name=f"s{m}")
                nc.scalar.activation(s_t[:], d_t[:], AF.Sigmoid)
                t_t = P5.tile([128, 512], F32, tag="gt3", bufs=2, name=f"tt{m}")
                nc.vector.tensor_sub(t_t[:], outs["vp"][:], outs["vn"][:])
                u_t = P5.tile([128, 512], F32, tag="gt4", bufs=2, name=f"ut{m}")
                nc.vector.tensor_mul(u_t[:], t_t[:], s_t[:])
                o_t = P5.tile([128, 512], F32, tag="gt5", bufs=2, name=f"ot{m}")
                nc.vector.tensor_add(o_t[:], u_t[:], outs["vn"][:])
                nc.sync.dma_start(d_outT[m * 128 : (m + 1) * 128, :], o_t[:])

    nc.compile()
    _cache["nc"] = nc
    return nc


def _tile_mk(W):
    """[D,D] -> [MT, MT, 128, 128]: block (m,k) = W[k*128:+128, m*128:+128]."""
    W = np.ascontiguousarray(W.reshape(MT, 128, MT, 128).transpose(2, 0, 1, 3))
    return W


def _tile_v(W):
    """[D,D] -> [2, MT, 128, 512]: block (nh,k) = W[k*128:+128, nh*512:+512]."""
    return np.ascontiguousarray(W.reshape(MT, 128, 2, 512).transpose(2, 0, 1, 3))


def _host_prep(inputs):
    i = {k: np.asarray(v, dtype=np.float32) for k, v in inputs.items()}
    SCALE = np.float32(1.0 / np.sqrt(DK))
    g = (i["fw3"] @ i["fw2"] @ i["fw1"])[0]  # [16]
    C = float(
        (i["fw3"] @ i["fw2"] @ i["fb1"] + i["fw3"] @ i["fb2"] + i["fb3"]).item()
    )
    svec = np.repeat(g, DK).astype(np.float32) * SCALE  # [1024]

    shared = {
        "negc": np.full((128, 1), -C, dtype=np.float32),
        "ones": np.ones(128, dtype=np.float32),
        "wqg": _tile_mk(i["Wq"] * svec[None, :]),
        "wk": _tile_mk(i["Wk"]),
        "pwq": _tile_mk(i["p_Wq"] * SCALE),
        "pwk": _tile_mk(i["p_Wk"]),
        "nwq": _tile_mk(i["n_Wq"] * SCALE),
        "nwk": _tile_mk(i["n_Wk"]),
        "pwo": _tile_mk(i["p_Wo"]),
        "nwo": _tile_mk(i["n_Wo"]),
        "wvp": _tile_mk(i["Wvp"]),
        "wvn": _tile_mk(i["Wvn"]),
        "wgp": _tile_mk(i["Wgp"]),
        "pwv": _tile_v(i["p_Wv"]),
        "nwv": _tile_v(i["n_Wv"]),
        "bqg": i["bq"] * svec,
        "bk": i["bk"],
        "pbq": i["p_bq"] * SCALE,
        "pbk": i["p_bk"],
        "pbo": i["p_bo"],
        "nbq": i["n_bq"] * SCALE,
        "nbk": i["n_bk"],
        "nbo": i["n_bo"],
        "bvp": i["bvp"],
        "bvn": i["bvn"],
        "bgp": i["bgp"],
        "pbvr": i["p_bv"].reshape(1, D),
        "nbvr": i["n_bv"].reshape(1, D),
    }
    shared = {k: np.ascontiguousarray(v, dtype=np.float32) for k, v in shared.items()}

    in_maps = []
    for b in range(B):
        m = dict(shared)
        m["xT"] = np.ascontiguousarray(i["feature"][b].T)
        m["dmT"] = np.ascontiguousarray(i["data_mask"][b].T)
        in_maps.append(m)
    return in_maps


def kernel(**inputs):
    nc = _build()
    in_maps = _host_prep(inputs)
    res = run_bass_kernel_spmd(nc, in_maps, core_ids=list(range(B)))
    out = np.empty((B, L, D), dtype=np.float32)
    pm = np.empty((B, H, L, L), dtype=np.float32)
    nm = np.empty((B, H, L, L), dtype=np.float32)
    for b in range(B):
        out[b] = res.results[b]["outT"].T
        pm[b] = res.results[b]["pmT"].transpose(0, 2, 1)
        nm[b] = res.results[b]["nmT"].transpose(0, 2, 1)
    return out, pm, nm
